# revision 1
# baseline (speedup 1.0000x reference)
"""BertSelfAttention on 8 Trainium2 NeuronCores (Bass/Tile).

Sharding: data-parallel over batch (B=2) x tensor-parallel over heads
(16 heads -> 4 groups of 4). Core c handles batch c//4, head group c%4,
holding column shards of Wq/Wk/Wv. No collectives.

All matmuls use float32r operands (TF32-class 1+8+11-bit, single PE pass,
4x the fp32 LOW_HIGH throughput; ~1.5e-4 per-matmul rel err). f32r data is
produced only by cast-DMAs (SWDGE) and ScalarE copies - the only legal
f32r producers. PSUM accumulation stays fp32 and the output path is fp32.

Per-core math (S=2048, group of 4 heads, d=64):
  xT   [1024, 2048]  host-pretransposed hidden states (this batch)
  QT/KT = W^T x^T + b   [256, 2048] f32r, d' on partitions (bias applied
          per-partition for free by the ScalarE evacuation ACTIVATE)
  V     = x W_aug + b_aug (rank-1 bias)      [2048, 260] f32r, head-major
          (65 cols/head: 64 V dims + ones column -> softmax denominators)
  per pair of heads, q-chunk-pair (2x512), k-tile (128):
    S^T[k,q] = sum_d KT[d,k] QT[d,q]   row-packed pairs (2x64 contraction);
                                       each stationary serves 2 q-chunks
    E = exp(S^T / 8)                   ScalarE, PSUM -> SBUF f32r, N=1024
    ctx^T[r,q] += sum_k Vaug[k,r] E[k,q]   r=0..63 ctx, r=64 denominator
  out_raw [260, 2048] = 4 heads x (64 ctx rows + 1 sums row)

Host unshards: out[b, :, g*256 + 64h + r] = (ctx_h / sums_h).T
"""

import sys

sys.path.insert(0, "/opt/trn_rl_repo")

import numpy as np

import concourse.bass as bass
import concourse.mybir as mybir
import concourse.tile as tile
from concourse import bacc
from concourse import bass_utils as _bass_utils
from concourse.bass_utils import run_bass_kernel_spmd

# The stock compile pipeline pins --enable-ldw-opt=false, which leaves every
# matmul's LDWEIGHTS serialized with its MATMUL (~140ns per matmul of pure
# PE stall). Re-enable the walrus LDW scheduling optimisation.
if not getattr(_bass_utils, "_ldw_opt_patched", False):
    _orig_run_command = _bass_utils.run_command

    def _run_command_ldwopt(argv, **kwargs):
        argv = [
            "--enable-ldw-opt=true" if a == "--enable-ldw-opt=false" else a
            for a in argv
        ]
        return _orig_run_command(argv, **kwargs)

    _bass_utils.run_command = _run_command_ldwopt
    _bass_utils._ldw_opt_patched = True

F32 = mybir.dt.float32
F32R = mybir.dt.float32r

HIDDEN = 1024
NUM_HEADS = 16
HEAD = 64
B, S = 2, 2048
N_CORES = 8
GROUPS = 4                      # head groups (tensor parallel)
HG = NUM_HEADS // GROUPS        # heads per group = 4
DG = HG * HEAD                  # 256 cols per group
KT_TILES = HIDDEN // 128        # 8 contraction tiles for projections
ST_TILES = S // 128             # 16 sequence tiles
QC = 512                        # q chunk width
N_QC = S // QC                  # 4
VAUG = HG * (HEAD + 1)          # 260: [V_h | ones] per head


def _build_kernel():
    nc = bacc.Bacc("TRN2")

    xT = nc.dram_tensor("xT", [HIDDEN, S], F32, kind="ExternalInput")
    # wq | wk packed: one big cast-DMA instead of two (SWDGE fixed costs)
    wqk = nc.dram_tensor("wqk", [HIDDEN, 2 * DG], F32, kind="ExternalInput")
    # wv pre-augmented on host: per head 64 cols + a zero col -> [1024, 260]
    wv = nc.dram_tensor("wv", [HIDDEN, VAUG], F32, kind="ExternalInput")
    # per-partition bias cols: bq[0:128], bq[128:], bk[0:128], bk[128:]
    bqk = nc.dram_tensor("bqk", [128, 4], F32, kind="ExternalInput")
    # bv interleaved with 1.0 at each head's ones column [1, 260]
    bv_aug = nc.dram_tensor("bv_aug", [1, VAUG], F32, kind="ExternalInput")
    ones_in = nc.dram_tensor("ones_in", [1, QC], F32, kind="ExternalInput")
    out_raw = nc.dram_tensor("out_raw", [VAUG, S], F32, kind="ExternalOutput")

    with tile.TileContext(nc) as tc:
        with (
            tc.tile_pool(name="consts", bufs=1) as consts,
            tc.tile_pool(name="work", bufs=4) as work,
            tc.tile_pool(name="outp", bufs=4) as outp,
            # 4 one-bank slots: QK-projection accumulators, then ctx accums
            tc.tile_pool(name="ps_b1", bufs=4, space="PSUM") as ps_b1,
            # 2 two-bank slots: V-projection psum, then score tiles
            tc.tile_pool(name="ps_b2", bufs=2, space="PSUM") as ps_b2,
        ):
            # ---- load inputs (cast to f32r in-flight via SWDGE) ----
            # order: what the first projection matmuls need comes first
            wqk_sb = consts.tile([128, KT_TILES, 2 * DG], F32R)
            nc.gpsimd.dma_start(
                wqk_sb[:], wqk.rearrange("(ko p) d -> p ko d", p=128)
            )
            wq_sb = wqk_sb[:, :, 0:DG]
            wk_sb = wqk_sb[:, :, DG:2 * DG]
            # xT in per-kt chunks so compute can start early
            xT_sb = consts.tile([128, KT_TILES, S], F32R)
            xT_r = xT.rearrange("(ko p) s -> p ko s", p=128)
            for kt in range(KT_TILES):
                nc.gpsimd.dma_start(xT_sb[:, kt, :], xT_r[:, kt, :])
            wv_sb = consts.tile([128, KT_TILES, VAUG], F32R)
            nc.gpsimd.dma_start(
                wv_sb[:], wv.rearrange("(ko p) d -> p ko d", p=128)
            )
            # bias rows are only needed at the tail of each projection
            bqk_sb = consts.tile([128, 4], F32)
            nc.sync.dma_start(bqk_sb[:], bqk[:])
            ones_sb = consts.tile([1, QC], F32R)
            nc.gpsimd.dma_start(ones_sb[:], ones_in[:])
            bvaug_sb = consts.tile([1, VAUG], F32R)
            nc.gpsimd.dma_start(bvaug_sb[:], bv_aug[:])

            QT_sb = consts.tile([128, 2, S], F32R)
            KT_sb = consts.tile([128, 2, S], F32R)
            v_sb = consts.tile([128, ST_TILES, VAUG], F32R)

            def qk_proj(m, use_b2=False):
                # kt-outer / sc-inner: each weight tile loaded once into the
                # PE, reused across the 4 moving chunks.
                for dst, w_sb, bcol in ((QT_sb, wq_sb, 0), (KT_sb, wk_sb, 2)):
                    if use_b2:
                        slots = [
                            ps_b2.tile([128, 1024], F32, tag="b2", name=f"qkb2{t}")
                            for t in range(2)
                        ]
                        pss = [
                            slots[sc // 2][:, (sc % 2) * 512:(sc % 2 + 1) * 512]
                            for sc in range(N_QC)
                        ]
                    else:
                        pss = [
                            ps_b1.tile([128, 512], F32, tag="b1", name=f"ps_proj{sc}")
                            for sc in range(N_QC)
                        ]
                    for kt in range(KT_TILES):
                        for sc in range(N_QC):
                            nc.tensor.matmul(
                                pss[sc][:],
                                w_sb[:, kt, m * 128:(m + 1) * 128],
                                xT_sb[:, kt, sc * QC:(sc + 1) * QC],
                                start=(kt == 0),
                                stop=(kt == KT_TILES - 1),
                            )
                    for sc in range(N_QC):
                        # bias[d'] is per-partition here: ACT applies it for
                        # free during the evacuation (out = in*1 + bias)
                        nc.scalar.activation(
                            dst[:, m, sc * QC:(sc + 1) * QC],
                            pss[sc][:],
                            mybir.ActivationFunctionType.Identity,
                            bias=bqk_sb[:, bcol + m:bcol + m + 1],
                        )

            def v_proj(st):
                psv = ps_b2.tile([128, 1024], F32, tag="b2", name="ps_v")
                for kt in range(KT_TILES):
                    nc.tensor.matmul(
                        psv[:, 0:VAUG],
                        xT_sb[:, kt, st * 128:(st + 1) * 128],
                        wv_sb[:, kt, :],
                        start=(kt == 0),
                        stop=False,
                    )
                # bias (and the per-head ones columns) as a rank-1 update
                nc.tensor.matmul(
                    psv[:, 0:VAUG],
                    ones_sb[:, 0:128],
                    bvaug_sb[:, :],
                    start=False,
                    stop=True,
                )
                nc.scalar.copy(v_sb[:, st, :], psv[:, 0:VAUG])

            def attn_group(p, qq, interleave_v=False):
                # Two q-chunks per pass: each stationary (KT/V slice) serves
                # two moving chunks -> half the LDWEIGHTS.
                qcs = (2 * qq, 2 * qq + 1)
                ctxs = {
                    (j, hh): ps_b1.tile(
                        [65, 512], F32, tag="b1", name=f"ctx{j}{hh}"
                    )
                    for j in range(2)
                    for hh in range(2)
                }
                for kt in range(ST_TILES):
                    if interleave_v:
                        v_proj(kt)  # ctx at kt consumes exactly V tile kt
                    sscs = [
                        ps_b2.tile([128, 1024], F32, tag="b2", name=f"ssc{j}")
                        for j in range(2)
                    ]
                    for hh in range(2):  # same KT slice for both chunks
                        rows = slice(hh * 64, hh * 64 + 64)
                        for j, qc in enumerate(qcs):
                            nc.tensor.matmul(
                                sscs[j][:, hh * 512:(hh + 1) * 512],
                                KT_sb[rows, p, kt * 128:(kt + 1) * 128],
                                QT_sb[rows, p, qc * QC:(qc + 1) * QC],
                                start=True,
                                stop=True,
                            )
                    ess = []
                    for j in range(2):
                        es = work.tile(
                            [128, 1024], F32R, tag="es", name=f"es{j}"
                        )
                        nc.scalar.activation(
                            es[:],
                            sscs[j][:],
                            mybir.ActivationFunctionType.Exp,
                            scale=0.125,
                        )
                        ess.append(es)
                    for hh in range(2):  # same V slice for both chunks
                        h = 2 * p + hh
                        for j in range(2):
                            nc.tensor.matmul(
                                ctxs[(j, hh)][:],
                                v_sb[:, kt, h * 65:(h + 1) * 65],
                                ess[j][:, hh * 512:(hh + 1) * 512],
                                start=(kt == 0),
                                stop=(kt == ST_TILES - 1),
                            )
                for j, qc in enumerate(qcs):
                    for hh in range(2):
                        h = 2 * p + hh
                        ctx_sb = outp.tile(
                            [65, 512], F32, tag="o", name="ctx_sb"
                        )
                        nc.vector.tensor_copy(
                            out=ctx_sb[:], in_=ctxs[(j, hh)][:]
                        )
                        nc.sync.dma_start(
                            out_raw[
                                h * 65:(h + 1) * 65,
                                qc * QC:(qc + 1) * QC,
                            ],
                            ctx_sb[:],
                        )

            qk_proj(0)
            qk_proj(1)
            for st in range(ST_TILES):
                v_proj(st)
            attn_group(0, 0)
            attn_group(0, 1)
            attn_group(1, 0)
            attn_group(1, 1)
    nc.compile()
    return nc


_NC_CACHE = None


def _get_nc():
    global _NC_CACHE
    if _NC_CACHE is None:
        _NC_CACHE = _build_kernel()
    return _NC_CACHE


def _prep_core_inputs(hidden_states, Wq, bq, Wk, bk, Wv, bv):
    """Host-side sharding: returns list of 8 in_maps."""
    xTs = [np.ascontiguousarray(hidden_states[b].T) for b in range(B)]
    in_maps = []
    for c in range(N_CORES):
        b, g = divmod(c, GROUPS)
        cs = slice(g * DG, (g + 1) * DG)
        wq_g = np.ascontiguousarray(Wq[:, cs])
        wk_g = np.ascontiguousarray(Wk[:, cs])
        wv_g = Wv[:, cs]
        bq_g, bk_g, bv_g = bq[cs], bk[cs], bv[cs]

        wv_aug = np.zeros((HIDDEN, VAUG), dtype=np.float32)
        bv_aug = np.zeros((1, VAUG), dtype=np.float32)
        for h in range(HG):
            wv_aug[:, h * 65:h * 65 + 64] = wv_g[:, h * 64:(h + 1) * 64]
            bv_aug[0, h * 65:h * 65 + 64] = bv_g[h * 64:(h + 1) * 64]
            bv_aug[0, h * 65 + 64] = 1.0

        bqk = np.stack(
            [bq_g[:128], bq_g[128:], bk_g[:128], bk_g[128:]], axis=1
        ).astype(np.float32)

        in_maps.append(
            {
                "xT": xTs[b],
                "wqk": np.ascontiguousarray(
                    np.concatenate([wq_g, wk_g], axis=1)
                ).astype(np.float32),
                "wv": np.ascontiguousarray(wv_aug),
                "bqk": np.ascontiguousarray(bqk),
                "bv_aug": bv_aug,
                "ones_in": np.ones((1, QC), dtype=np.float32),
            }
        )
    return in_maps


def _unshard(results):
    out = np.empty((B, S, HIDDEN), dtype=np.float32)
    for c in range(N_CORES):
        b, g = divmod(c, GROUPS)
        raw = results[c]["out_raw"]  # [260, 2048]
        for h in range(HG):
            ctx = raw[h * 65:h * 65 + 64]          # [64, S]
            sums = raw[h * 65 + 64]                # [S]
            col0 = g * DG + h * HEAD
            out[b, :, col0:col0 + HEAD] = (ctx / sums).T
    return out


def kernel(**inputs):
    inputs = {k: np.asarray(v, dtype=np.float32) for k, v in inputs.items()}
    nc = _get_nc()
    in_maps = _prep_core_inputs(**inputs)
    res = run_bass_kernel_spmd(nc, in_maps, core_ids=list(range(N_CORES)))
    return _unshard(res.results)


if __name__ == "__main__":
    rng = np.random.default_rng(0)
    scale = 1.0 / np.sqrt(HIDDEN)
    ins = {
        "hidden_states": rng.standard_normal((B, S, HIDDEN), dtype=np.float32),
        "Wq": rng.standard_normal((HIDDEN, HIDDEN), dtype=np.float32) * scale,
        "bq": rng.standard_normal(HIDDEN, dtype=np.float32) * 0.01,
        "Wk": rng.standard_normal((HIDDEN, HIDDEN), dtype=np.float32) * scale,
        "bk": rng.standard_normal(HIDDEN, dtype=np.float32) * 0.01,
        "Wv": rng.standard_normal((HIDDEN, HIDDEN), dtype=np.float32) * scale,
        "bv": rng.standard_normal(HIDDEN, dtype=np.float32) * 0.01,
    }
    out = kernel(**ins)

    def ref(x, Wq, bq, Wk, bk, Wv, bv):
        q = (x @ Wq + bq).reshape(B, S, NUM_HEADS, HEAD).transpose(0, 2, 1, 3)
        k = (x @ Wk + bk).reshape(B, S, NUM_HEADS, HEAD).transpose(0, 2, 1, 3)
        v = (x @ Wv + bv).reshape(B, S, NUM_HEADS, HEAD).transpose(0, 2, 1, 3)
        s = np.einsum("bhqd,bhkd->bhqk", q, k) / np.sqrt(HEAD)
        s = s - s.max(-1, keepdims=True)
        p = np.exp(s)
        p /= p.sum(-1, keepdims=True)
        c = np.einsum("bhqk,bhkd->bhqd", p, v)
        return c.transpose(0, 2, 1, 3).reshape(B, S, HIDDEN)

    exp = ref(
        ins["hidden_states"].astype(np.float64),
        ins["Wq"].astype(np.float64), ins["bq"].astype(np.float64),
        ins["Wk"].astype(np.float64), ins["bk"].astype(np.float64),
        ins["Wv"].astype(np.float64), ins["bv"].astype(np.float64),
    )
    print("L2 rel err:", np.linalg.norm(out - exp) / np.linalg.norm(exp))
    print("max abs err:", np.abs(out - exp).max())



# revision 5
# speedup vs baseline: 1.2442x; 1.2442x over previous
"""BertSelfAttention on 8 Trainium2 NeuronCores (Bass/Tile).

Sharding: data-parallel over batch (B=2) x tensor-parallel over heads
(16 heads -> 4 groups of 4). Core c handles batch c//4, head group c%4,
holding column shards of Wq/Wk/Wv. No collectives.

v2 design (vs f32r baseline at 251us):
  * All matmul operands are bf16 (host-pre-cast; fp32/f32r moving operands
    stream at 2 cycles/column, bf16 at 1 -> 2x PE throughput; also halves
    input DMA bytes). PSUM accumulation stays fp32.
  * ScalarE does nothing but the 16.7M-element exp (its hard floor,
    ~1ns/elem/lane); all PSUM evacuations moved to VectorE
    (tensor_scalar_add applies the QK bias during evacuation).
  * Attention runs as 8 passes of (head-pair p, 512-wide q chunk qc);
    per kt: one [128,1024] score PSUM tile (2 row-packed 64-contraction
    matmuls, concurrent via PE row tiling), one [128,1024] exp ACT, two
    [65,512] ctx accumulations. Ctx matmuls lag one kt so the PE never
    waits on the exp it just requested.
  * V projection (pass 0) and the m=1 QK projection (passes 1-2, 2 MMs
    per kt) are interleaved into the attention loop, filling the PE's
    slack while ScalarE stays exp-saturated.
  * PSUM budget: scores 2x2 banks + ctx 2x1 + interleaved-proj 2x1 = 8.

Per-core layout (S=2048, 4 heads, d=64):
  xT    [1024, 2048] bf16   hidden states, this batch, pretransposed
  QT/KT [128, 2, 2048] bf16 d' on partitions (m in {0,1} = head pair)
  V     [2048, 260] bf16    head-major, 65 cols/head: 64 V dims + ones
                            column -> softmax denominators ride along
  out_raw [260, 2048] f32   4 heads x (64 ctx rows + 1 sums row)

Host unshards: out[b, :, g*256 + 64h + r] = (ctx_h / sums_h).T
"""

import sys

sys.path.insert(0, "/opt/trn_rl_repo")

import ml_dtypes
import numpy as np

import concourse.bass as bass
import concourse.mybir as mybir
import concourse.tile as tile
from concourse import bacc
from concourse import bass_utils as _bass_utils
from concourse.bass_utils import run_bass_kernel_spmd

# NOTE: the walrus --enable-ldw-opt=true rewrite (used by the f32r baseline)
# rejects bf16 LDWEIGHTS ("InstLdweights is not compatible with LDW
# optimization" — bf16 triggers the FWL weight-load path). Stock flags; the
# PE's 64-deep reorder window still pulls LDWEIGHTS ahead in silicon.

F32 = mybir.dt.float32
BF16 = mybir.dt.bfloat16
NP_BF16 = ml_dtypes.bfloat16

HIDDEN = 1024
NUM_HEADS = 16
HEAD = 64
B, S = 2, 2048
N_CORES = 8
GROUPS = 4                      # head groups (tensor parallel)
HG = NUM_HEADS // GROUPS        # heads per group = 4
DG = HG * HEAD                  # 256 cols per group
KT_TILES = HIDDEN // 128        # 8 contraction tiles for projections
ST_TILES = S // 128             # 16 sequence tiles
QC = 512                        # q chunk width
N_QC = S // QC                  # 4
VAUG = HG * (HEAD + 1)          # 260: [V_h | ones] per head


def _build_kernel():
    nc = bacc.Bacc("TRN2")

    xT = nc.dram_tensor("xT", [HIDDEN, S], BF16, kind="ExternalInput")
    # packed [wq_m0 | wk_m0 | wq_m1 | wk_m1], 128 cols each
    wqk = nc.dram_tensor("wqk", [HIDDEN, 2 * DG], BF16, kind="ExternalInput")
    # wv pre-augmented on host: per head 64 cols + a zero col -> [1024, 260]
    wv = nc.dram_tensor("wv", [HIDDEN, VAUG], BF16, kind="ExternalInput")
    # per-partition bias cols: bq_m0, bq_m1, bk_m0, bk_m1
    bqk = nc.dram_tensor("bqk", [128, 4], F32, kind="ExternalInput")
    # bv interleaved with 1.0 at each head's ones column [1, 260]
    bv_aug = nc.dram_tensor("bv_aug", [1, VAUG], BF16, kind="ExternalInput")
    ones_in = nc.dram_tensor("ones_in", [1, 128], BF16, kind="ExternalInput")
    out_raw = nc.dram_tensor("out_raw", [VAUG, S], F32, kind="ExternalOutput")

    with tile.TileContext(nc) as tc:
        with (
            tc.tile_pool(name="consts", bufs=1) as consts,
            tc.tile_pool(name="work", bufs=4) as work,
            tc.tile_pool(name="outp", bufs=4) as outp,
            # 2-bank slots: lead -> Q-proj accumulators; attention -> scores
            tc.tile_pool(name="ps2", bufs=2, space="PSUM") as ps2,
            # 1-bank slots: lead -> K-proj accums sc0/1; attention -> ctx
            tc.tile_pool(name="psA", bufs=2, space="PSUM") as psA,
            # 1-bank slots: lead -> K-proj accums sc2/3; attention -> v/qk1
            tc.tile_pool(name="psB", bufs=2, space="PSUM") as psB,
        ):
            # ---- input DMAs (bf16, HWDGE), compute-critical first ----
            wqk_r = wqk.rearrange("(ko p) d -> p ko d", p=128)
            wqk_sb = consts.tile([128, KT_TILES, 2 * DG], BF16)
            nc.sync.dma_start(wqk_sb[:, :, 0:DG], wqk_r[:, :, 0:DG])
            xT_sb = consts.tile([128, KT_TILES, S], BF16)
            xT_r = xT.rearrange("(ko p) s -> p ko s", p=128)
            for kt in range(KT_TILES):
                nc.sync.dma_start(xT_sb[:, kt, :], xT_r[:, kt, :])
            wv_sb = consts.tile([128, KT_TILES, VAUG], BF16)
            nc.sync.dma_start(
                wv_sb[:], wv.rearrange("(ko p) d -> p ko d", p=128)
            )
            nc.sync.dma_start(wqk_sb[:, :, DG:2 * DG], wqk_r[:, :, DG:2 * DG])
            bqk_sb = consts.tile([128, 4], F32)
            nc.sync.dma_start(bqk_sb[:], bqk[:])
            ones_sb = consts.tile([1, 128], BF16)
            nc.sync.dma_start(ones_sb[:], ones_in[:])
            bvaug_sb = consts.tile([1, VAUG], BF16)
            nc.sync.dma_start(bvaug_sb[:], bv_aug[:])

            QT_sb = consts.tile([128, 2, S], BF16)
            KT_sb = consts.tile([128, 2, S], BF16)
            v_sb = consts.tile([128, ST_TILES, VAUG], BF16)

            # wq_m at cols m*256..m*256+128, wk_m at m*256+128..m*256+256
            def w_slice(w, m, ko):
                c0 = m * 2 * 128 + w * 128
                return wqk_sb[:, ko, c0:c0 + 128]

            def qk_evac(dst, ps, w, m, sc):
                # bias[d'] is per-partition: DVE adds it during evacuation
                nc.vector.tensor_scalar_add(
                    dst[:, m, sc * QC:(sc + 1) * QC],
                    ps,
                    bqk_sb[:, w * 2 + m:w * 2 + m + 1],
                )

            # ---- lead: QK projection for m=0 (Q and K share each xT
            # chunk as it lands; kt-outer keeps the weight stationary) ----
            qslots = [ps2.tile([128, 1024], F32, tag="s2", name=f"q0_{t}")
                      for t in range(2)]
            qps = [qslots[sc // 2][:, (sc % 2) * QC:(sc % 2 + 1) * QC]
                   for sc in range(N_QC)]
            kps = [psA.tile([128, QC], F32, tag="sA", name=f"k0a{t}")
                   for t in range(2)]
            kps += [psB.tile([128, QC], F32, tag="sB", name=f"k0b{t}")
                    for t in range(2)]
            for ko in range(KT_TILES):
                st = (ko == 0)
                sp = (ko == KT_TILES - 1)
                for sc in range(N_QC):
                    nc.tensor.matmul(
                        qps[sc][:], w_slice(0, 0, ko),
                        xT_sb[:, ko, sc * QC:(sc + 1) * QC],
                        start=st, stop=sp,
                    )
                for sc in range(N_QC):
                    nc.tensor.matmul(
                        kps[sc][:], w_slice(1, 0, ko),
                        xT_sb[:, ko, sc * QC:(sc + 1) * QC],
                        start=st, stop=sp,
                    )
            for sc in range(N_QC):
                qk_evac(QT_sb, qps[sc][:], 0, 0, sc)
            for sc in range(N_QC):
                qk_evac(KT_sb, kps[sc][:], 1, 0, sc)

            def v_proj(st):
                psv = psB.tile([128, QC], F32, tag="sB", name="ps_v")
                for ko in range(KT_TILES):
                    nc.tensor.matmul(
                        psv[:, 0:VAUG],
                        xT_sb[:, ko, st * 128:(st + 1) * 128],
                        wv_sb[:, ko, :],
                        start=(ko == 0), stop=False,
                    )
                # bias (and the per-head ones columns) as a rank-1 update
                nc.tensor.matmul(
                    psv[:, 0:VAUG], ones_sb[:, :], bvaug_sb[:, :],
                    start=False, stop=True,
                )
                nc.vector.tensor_copy(out=v_sb[:, st, :], in_=psv[:, 0:VAUG])

            # m=1 QK projection as 64 MMs dribbled into passes 1-2
            qk1_flat = [(w, sc, ko)
                        for w in range(2)
                        for sc in range(N_QC)
                        for ko in range(KT_TILES)]
            qk1_state = {"ps": None}

            def qk1_step(i):
                w, sc, ko = qk1_flat[i]
                if ko == 0:
                    qk1_state["ps"] = psB.tile(
                        [128, QC], F32, tag="sB", name=f"qk1_{w}{sc}"
                    )
                ps = qk1_state["ps"]
                nc.tensor.matmul(
                    ps[:], w_slice(w, 1, ko),
                    xT_sb[:, ko, sc * QC:(sc + 1) * QC],
                    start=(ko == 0), stop=(ko == KT_TILES - 1),
                )
                if ko == KT_TILES - 1:
                    qk_evac(QT_sb if w == 0 else KT_sb, ps[:], w, 1, sc)

            def attn_pass(pi, p, qc):
                ctxs = [
                    psA.tile([65, QC], F32, tag="sA", name=f"ctx{hh}")
                    for hh in range(2)
                ]

                def ctx_mms(es, kt):
                    for hh in range(2):
                        h = 2 * p + hh
                        nc.tensor.matmul(
                            ctxs[hh][:],
                            v_sb[:, kt, h * 65:(h + 1) * 65],
                            es[:, hh * QC:(hh + 1) * QC],
                            start=(kt == 0), stop=(kt == ST_TILES - 1),
                        )

                prev = None
                for kt in range(ST_TILES):
                    if pi == 0:
                        v_proj(kt)  # ctx at kt consumes exactly V tile kt
                    elif pi in (1, 2):
                        base = (pi - 1) * 32 + kt * 2
                        qk1_step(base)
                        qk1_step(base + 1)
                    ssc = ps2.tile([128, 1024], F32, tag="s2", name="ssc")
                    for hh in range(2):  # row-packed pair, runs concurrent
                        rows = slice(hh * 64, hh * 64 + 64)
                        nc.tensor.matmul(
                            ssc[:, hh * QC:(hh + 1) * QC],
                            KT_sb[rows, p, kt * 128:(kt + 1) * 128],
                            QT_sb[rows, p, qc * QC:(qc + 1) * QC],
                            start=True, stop=True,
                        )
                    es = work.tile([128, 1024], BF16, tag="es", name="es")
                    nc.scalar.activation(
                        es[:], ssc[:],
                        mybir.ActivationFunctionType.Exp,
                        scale=0.125,
                    )
                    if prev is not None:
                        ctx_mms(*prev)  # lag 1 kt: never stall on fresh exp
                    prev = (es, kt)
                ctx_mms(*prev)
                for hh in range(2):
                    h = 2 * p + hh
                    ctx_sb = outp.tile([65, QC], F32, tag="o", name="ctx_sb")
                    nc.vector.tensor_copy(out=ctx_sb[:], in_=ctxs[hh][:])
                    nc.sync.dma_start(
                        out_raw[h * 65:(h + 1) * 65, qc * QC:(qc + 1) * QC],
                        ctx_sb[:],
                    )

            for pi, (p, qc) in enumerate(
                [(0, 0), (0, 1), (0, 2), (0, 3),
                 (1, 0), (1, 1), (1, 2), (1, 3)]
            ):
                attn_pass(pi, p, qc)
    nc.compile()
    return nc


_NC_CACHE = None


def _get_nc():
    global _NC_CACHE
    if _NC_CACHE is None:
        _NC_CACHE = _build_kernel()
    return _NC_CACHE


def _prep_core_inputs(hidden_states, Wq, bq, Wk, bk, Wv, bv):
    """Host-side sharding: returns list of 8 in_maps (bf16 pre-cast)."""
    xTs = [
        np.ascontiguousarray(hidden_states[b].T).astype(NP_BF16)
        for b in range(B)
    ]
    in_maps = []
    for c in range(N_CORES):
        b, g = divmod(c, GROUPS)
        cs = slice(g * DG, (g + 1) * DG)
        wq_g, wk_g, wv_g = Wq[:, cs], Wk[:, cs], Wv[:, cs]
        bq_g, bk_g, bv_g = bq[cs], bk[cs], bv[cs]

        # [wq_m0 | wk_m0 | wq_m1 | wk_m1]
        wqk_p = np.concatenate(
            [wq_g[:, 0:128], wk_g[:, 0:128], wq_g[:, 128:256],
             wk_g[:, 128:256]], axis=1,
        )

        wv_aug = np.zeros((HIDDEN, VAUG), dtype=np.float32)
        bv_aug = np.zeros((1, VAUG), dtype=np.float32)
        for h in range(HG):
            wv_aug[:, h * 65:h * 65 + 64] = wv_g[:, h * 64:(h + 1) * 64]
            bv_aug[0, h * 65:h * 65 + 64] = bv_g[h * 64:(h + 1) * 64]
            bv_aug[0, h * 65 + 64] = 1.0

        bqk = np.stack(
            [bq_g[:128], bq_g[128:], bk_g[:128], bk_g[128:]], axis=1
        ).astype(np.float32)

        in_maps.append(
            {
                "xT": xTs[b],
                "wqk": np.ascontiguousarray(wqk_p).astype(NP_BF16),
                "wv": wv_aug.astype(NP_BF16),
                "bqk": np.ascontiguousarray(bqk),
                "bv_aug": bv_aug.astype(NP_BF16),
                "ones_in": np.ones((1, 128), dtype=NP_BF16),
            }
        )
    return in_maps


def _unshard(results):
    out = np.empty((B, S, HIDDEN), dtype=np.float32)
    for c in range(N_CORES):
        b, g = divmod(c, GROUPS)
        raw = results[c]["out_raw"]  # [260, 2048]
        for h in range(HG):
            ctx = raw[h * 65:h * 65 + 64]          # [64, S]
            sums = raw[h * 65 + 64]                # [S]
            col0 = g * DG + h * HEAD
            out[b, :, col0:col0 + HEAD] = (ctx / sums).T
    return out


def kernel(**inputs):
    inputs = {k: np.asarray(v, dtype=np.float32) for k, v in inputs.items()}
    nc = _get_nc()
    in_maps = _prep_core_inputs(**inputs)
    res = run_bass_kernel_spmd(nc, in_maps, core_ids=list(range(N_CORES)))
    return _unshard(res.results)


if __name__ == "__main__":
    rng = np.random.default_rng(0)
    scale = 1.0 / np.sqrt(HIDDEN)
    ins = {
        "hidden_states": rng.standard_normal((B, S, HIDDEN), dtype=np.float32),
        "Wq": rng.standard_normal((HIDDEN, HIDDEN), dtype=np.float32) * scale,
        "bq": rng.standard_normal(HIDDEN, dtype=np.float32) * 0.01,
        "Wk": rng.standard_normal((HIDDEN, HIDDEN), dtype=np.float32) * scale,
        "bk": rng.standard_normal(HIDDEN, dtype=np.float32) * 0.01,
        "Wv": rng.standard_normal((HIDDEN, HIDDEN), dtype=np.float32) * scale,
        "bv": rng.standard_normal(HIDDEN, dtype=np.float32) * 0.01,
    }
    out = kernel(**ins)

    def ref(x, Wq, bq, Wk, bk, Wv, bv):
        q = (x @ Wq + bq).reshape(B, S, NUM_HEADS, HEAD).transpose(0, 2, 1, 3)
        k = (x @ Wk + bk).reshape(B, S, NUM_HEADS, HEAD).transpose(0, 2, 1, 3)
        v = (x @ Wv + bv).reshape(B, S, NUM_HEADS, HEAD).transpose(0, 2, 1, 3)
        s = np.einsum("bhqd,bhkd->bhqk", q, k) / np.sqrt(HEAD)
        s = s - s.max(-1, keepdims=True)
        p = np.exp(s)
        p /= p.sum(-1, keepdims=True)
        c = np.einsum("bhqk,bhkd->bhqd", p, v)
        return c.transpose(0, 2, 1, 3).reshape(B, S, HIDDEN)

    exp = ref(
        ins["hidden_states"].astype(np.float64),
        ins["Wq"].astype(np.float64), ins["bq"].astype(np.float64),
        ins["Wk"].astype(np.float64), ins["bk"].astype(np.float64),
        ins["Wv"].astype(np.float64), ins["bv"].astype(np.float64),
    )
    print("L2 rel err:", np.linalg.norm(out - exp) / np.linalg.norm(exp))
    print("max abs err:", np.abs(out - exp).max())


# revision 6
# speedup vs baseline: 1.2746x; 1.0245x over previous
"""BertSelfAttention on 8 Trainium2 NeuronCores (Bass/Tile).

Sharding: data-parallel over batch (B=2) x tensor-parallel over heads
(16 heads -> 4 groups of 4). Core c handles batch c//4, head group c%4,
holding column shards of Wq/Wk/Wv. No collectives.

v2 design (vs f32r baseline at 251us):
  * All matmul operands are bf16 (host-pre-cast; fp32/f32r moving operands
    stream at 2 cycles/column, bf16 at 1 -> 2x PE throughput; also halves
    input DMA bytes). PSUM accumulation stays fp32.
  * ScalarE does nothing but the 16.7M-element exp (its hard floor,
    ~1ns/elem/lane); all PSUM evacuations moved to VectorE
    (tensor_scalar_add applies the QK bias during evacuation).
  * Attention runs as 8 passes of (head-pair p, 512-wide q chunk qc);
    per kt: one [128,1024] score PSUM tile (2 row-packed 64-contraction
    matmuls, concurrent via PE row tiling), one [128,1024] exp ACT, two
    [65,512] ctx accumulations. Ctx matmuls lag one kt so the PE never
    waits on the exp it just requested.
  * V projection (pass 0) and the m=1 QK projection (passes 1-2, 2 MMs
    per kt) are interleaved into the attention loop, filling the PE's
    slack while ScalarE stays exp-saturated.
  * PSUM budget: scores 2x2 banks + ctx 2x1 + interleaved-proj 2x1 = 8.

Per-core layout (S=2048, 4 heads, d=64):
  xT    [1024, 2048] bf16   hidden states, this batch, pretransposed
  QT/KT [128, 2, 2048] bf16 d' on partitions (m in {0,1} = head pair)
  V     [2048, 260] bf16    head-major, 65 cols/head: 64 V dims + ones
                            column -> softmax denominators ride along
  out_raw [260, 2048] f32   4 heads x (64 ctx rows + 1 sums row)

Host unshards: out[b, :, g*256 + 64h + r] = (ctx_h / sums_h).T
"""

import sys

sys.path.insert(0, "/opt/trn_rl_repo")

import ml_dtypes
import numpy as np

import concourse.bass as bass
import concourse.mybir as mybir
import concourse.tile as tile
from concourse import bacc
from concourse import bass_utils as _bass_utils
from concourse.bass_utils import run_bass_kernel_spmd

# NOTE: the walrus --enable-ldw-opt=true rewrite (used by the f32r baseline)
# rejects bf16 LDWEIGHTS ("InstLdweights is not compatible with LDW
# optimization" — bf16 triggers the FWL weight-load path). Stock flags; the
# PE's 64-deep reorder window still pulls LDWEIGHTS ahead in silicon.

F32 = mybir.dt.float32
BF16 = mybir.dt.bfloat16
NP_BF16 = ml_dtypes.bfloat16

HIDDEN = 1024
NUM_HEADS = 16
HEAD = 64
B, S = 2, 2048
N_CORES = 8
GROUPS = 4                      # head groups (tensor parallel)
HG = NUM_HEADS // GROUPS        # heads per group = 4
DG = HG * HEAD                  # 256 cols per group
KT_TILES = HIDDEN // 128        # 8 contraction tiles for projections
ST_TILES = S // 128             # 16 sequence tiles
QC = 512                        # q chunk width
N_QC = S // QC                  # 4
VAUG = HG * (HEAD + 1)          # 260: [V_h | ones] per head


def _build_kernel():
    nc = bacc.Bacc("TRN2")

    xT = nc.dram_tensor("xT", [HIDDEN, S], BF16, kind="ExternalInput")
    # packed [wq_m0 | wk_m0 | wq_m1 | wk_m1], 128 cols each
    wqk = nc.dram_tensor("wqk", [HIDDEN, 2 * DG], BF16, kind="ExternalInput")
    # wv pre-augmented on host: per head 64 cols + a zero col -> [1024, 260]
    wv = nc.dram_tensor("wv", [HIDDEN, VAUG], BF16, kind="ExternalInput")
    # per-partition bias cols: bq_m0, bq_m1, bk_m0, bk_m1
    bqk = nc.dram_tensor("bqk", [128, 4], F32, kind="ExternalInput")
    # bv (+1.0 at each head's ones column), host-replicated to all
    # partitions so DVE can apply it elementwise during V evacuation
    bv_aug = nc.dram_tensor("bv_aug", [128, VAUG], BF16, kind="ExternalInput")
    out_raw = nc.dram_tensor("out_raw", [VAUG, S], F32, kind="ExternalOutput")

    with tile.TileContext(nc) as tc:
        with (
            tc.tile_pool(name="consts", bufs=1) as consts,
            tc.tile_pool(name="work", bufs=4) as work,
            tc.tile_pool(name="outp", bufs=4) as outp,
            # 2-bank slots: lead -> Q-proj accumulators; attention -> scores
            tc.tile_pool(name="ps2", bufs=2, space="PSUM") as ps2,
            # 1-bank slots: lead -> K-proj accums sc0/1; attention -> ctx
            tc.tile_pool(name="psA", bufs=2, space="PSUM") as psA,
            # 1-bank slots: lead -> K-proj accums sc2/3; attention -> v/qk1
            tc.tile_pool(name="psB", bufs=2, space="PSUM") as psB,
        ):
            # ---- PE warmup: ~4us of dummy matmuls so the HAM clock
            # gate reaches 8/8 before the real projections start ----
            wu = consts.tile([128, 128], BF16)
            nc.vector.memset(wu[:], 0)
            wups = psB.tile([128, QC], F32, tag="sB", name="warmup")
            N_WARM = 40
            for i in range(N_WARM):
                nc.tensor.matmul(
                    wups[:, 0:128], wu[:], wu[:],
                    start=(i == 0), stop=(i == N_WARM - 1),
                )

            # ---- input DMAs (bf16, HWDGE), compute-critical first ----
            wqk_r = wqk.rearrange("(ko p) d -> p ko d", p=128)
            wqk_sb = consts.tile([128, KT_TILES, 2 * DG], BF16)
            nc.sync.dma_start(wqk_sb[:, :, 0:DG], wqk_r[:, :, 0:DG])
            xT_sb = consts.tile([128, KT_TILES, S], BF16)
            xT_r = xT.rearrange("(ko p) s -> p ko s", p=128)
            for kt in range(KT_TILES):
                nc.sync.dma_start(xT_sb[:, kt, :], xT_r[:, kt, :])
            wv_sb = consts.tile([128, KT_TILES, VAUG], BF16)
            nc.sync.dma_start(
                wv_sb[:], wv.rearrange("(ko p) d -> p ko d", p=128)
            )
            nc.sync.dma_start(wqk_sb[:, :, DG:2 * DG], wqk_r[:, :, DG:2 * DG])
            bqk_sb = consts.tile([128, 4], F32)
            nc.sync.dma_start(bqk_sb[:], bqk[:])
            bvaug_sb = consts.tile([128, VAUG], BF16)
            nc.sync.dma_start(bvaug_sb[:], bv_aug[:])

            QT_sb = consts.tile([128, 2, S], BF16)
            KT_sb = consts.tile([128, 2, S], BF16)
            v_sb = consts.tile([128, ST_TILES, VAUG], BF16)

            # wq_m at cols m*256..m*256+128, wk_m at m*256+128..m*256+256
            def w_slice(w, m, ko):
                c0 = m * 2 * 128 + w * 128
                return wqk_sb[:, ko, c0:c0 + 128]

            def qk_evac(dst, ps, w, m, sc):
                # bias[d'] is per-partition: DVE adds it during evacuation
                nc.vector.tensor_scalar_add(
                    dst[:, m, sc * QC:(sc + 1) * QC],
                    ps,
                    bqk_sb[:, w * 2 + m:w * 2 + m + 1],
                )

            # ---- lead: QK projection for m=0 (Q and K share each xT
            # chunk as it lands; kt-outer keeps the weight stationary) ----
            qslots = [ps2.tile([128, 1024], F32, tag="s2", name=f"q0_{t}")
                      for t in range(2)]
            qps = [qslots[sc // 2][:, (sc % 2) * QC:(sc % 2 + 1) * QC]
                   for sc in range(N_QC)]
            kps = [psA.tile([128, QC], F32, tag="sA", name=f"k0a{t}")
                   for t in range(2)]
            kps += [psB.tile([128, QC], F32, tag="sB", name=f"k0b{t}")
                    for t in range(2)]
            for ko in range(KT_TILES):
                st = (ko == 0)
                sp = (ko == KT_TILES - 1)
                for sc in range(N_QC):
                    nc.tensor.matmul(
                        qps[sc][:], w_slice(0, 0, ko),
                        xT_sb[:, ko, sc * QC:(sc + 1) * QC],
                        start=st, stop=sp,
                    )
                for sc in range(N_QC):
                    nc.tensor.matmul(
                        kps[sc][:], w_slice(1, 0, ko),
                        xT_sb[:, ko, sc * QC:(sc + 1) * QC],
                        start=st, stop=sp,
                    )
            for sc in range(N_QC):
                qk_evac(QT_sb, qps[sc][:], 0, 0, sc)
            for sc in range(N_QC):
                qk_evac(KT_sb, kps[sc][:], 1, 0, sc)

            def v_proj(st):
                psv = psB.tile([128, QC], F32, tag="sB", name="ps_v")
                for ko in range(KT_TILES):
                    nc.tensor.matmul(
                        psv[:, 0:VAUG],
                        xT_sb[:, ko, st * 128:(st + 1) * 128],
                        wv_sb[:, ko, :],
                        start=(ko == 0), stop=(ko == KT_TILES - 1),
                    )
                # bias + per-head ones columns land during evacuation
                nc.vector.tensor_add(
                    v_sb[:, st, :], psv[:, 0:VAUG], bvaug_sb[:]
                )

            # m=1 QK projection as 64 MMs dribbled into passes 1-2
            qk1_flat = [(w, sc, ko)
                        for w in range(2)
                        for sc in range(N_QC)
                        for ko in range(KT_TILES)]
            qk1_state = {"ps": None}

            def qk1_step(i):
                w, sc, ko = qk1_flat[i]
                if ko == 0:
                    qk1_state["ps"] = psB.tile(
                        [128, QC], F32, tag="sB", name=f"qk1_{w}{sc}"
                    )
                ps = qk1_state["ps"]
                nc.tensor.matmul(
                    ps[:], w_slice(w, 1, ko),
                    xT_sb[:, ko, sc * QC:(sc + 1) * QC],
                    start=(ko == 0), stop=(ko == KT_TILES - 1),
                )
                if ko == KT_TILES - 1:
                    qk_evac(QT_sb if w == 0 else KT_sb, ps[:], w, 1, sc)

            def attn_pass(pi, p, qc):
                ctxs = [
                    psA.tile([65, QC], F32, tag="sA", name=f"ctx{hh}")
                    for hh in range(2)
                ]

                def ctx_mms(es, kt):
                    for hh in range(2):
                        h = 2 * p + hh
                        nc.tensor.matmul(
                            ctxs[hh][:],
                            v_sb[:, kt, h * 65:(h + 1) * 65],
                            es[:, hh * QC:(hh + 1) * QC],
                            start=(kt == 0), stop=(kt == ST_TILES - 1),
                        )

                prev = None
                for kt in range(ST_TILES):
                    if pi == 0:
                        v_proj(kt)  # ctx at kt consumes exactly V tile kt
                    elif pi in (1, 2):
                        base = (pi - 1) * 32 + kt * 2
                        qk1_step(base)
                        qk1_step(base + 1)
                    ssc = ps2.tile([128, 1024], F32, tag="s2", name="ssc")
                    for hh in range(2):  # row-packed pair, runs concurrent
                        rows = slice(hh * 64, hh * 64 + 64)
                        nc.tensor.matmul(
                            ssc[:, hh * QC:(hh + 1) * QC],
                            KT_sb[rows, p, kt * 128:(kt + 1) * 128],
                            QT_sb[rows, p, qc * QC:(qc + 1) * QC],
                            start=True, stop=True,
                        )
                    es = work.tile([128, 1024], BF16, tag="es", name="es")
                    nc.scalar.activation(
                        es[:], ssc[:],
                        mybir.ActivationFunctionType.Exp,
                        scale=0.125,
                    )
                    if prev is not None:
                        ctx_mms(*prev)  # lag 1 kt: never stall on fresh exp
                    prev = (es, kt)
                ctx_mms(*prev)
                for hh in range(2):
                    h = 2 * p + hh
                    ctx_sb = outp.tile([65, QC], F32, tag="o", name="ctx_sb")
                    nc.vector.tensor_copy(out=ctx_sb[:], in_=ctxs[hh][:])
                    nc.sync.dma_start(
                        out_raw[h * 65:(h + 1) * 65, qc * QC:(qc + 1) * QC],
                        ctx_sb[:],
                    )

            for pi, (p, qc) in enumerate(
                [(0, 0), (0, 1), (0, 2), (0, 3),
                 (1, 0), (1, 1), (1, 2), (1, 3)]
            ):
                attn_pass(pi, p, qc)
    nc.compile()
    return nc


_NC_CACHE = None


def _get_nc():
    global _NC_CACHE
    if _NC_CACHE is None:
        _NC_CACHE = _build_kernel()
    return _NC_CACHE


def _prep_core_inputs(hidden_states, Wq, bq, Wk, bk, Wv, bv):
    """Host-side sharding: returns list of 8 in_maps (bf16 pre-cast)."""
    xTs = [
        np.ascontiguousarray(hidden_states[b].T).astype(NP_BF16)
        for b in range(B)
    ]
    in_maps = []
    for c in range(N_CORES):
        b, g = divmod(c, GROUPS)
        cs = slice(g * DG, (g + 1) * DG)
        wq_g, wk_g, wv_g = Wq[:, cs], Wk[:, cs], Wv[:, cs]
        bq_g, bk_g, bv_g = bq[cs], bk[cs], bv[cs]

        # [wq_m0 | wk_m0 | wq_m1 | wk_m1]
        wqk_p = np.concatenate(
            [wq_g[:, 0:128], wk_g[:, 0:128], wq_g[:, 128:256],
             wk_g[:, 128:256]], axis=1,
        )

        wv_aug = np.zeros((HIDDEN, VAUG), dtype=np.float32)
        bv_aug = np.zeros((1, VAUG), dtype=np.float32)  # replicated below
        for h in range(HG):
            wv_aug[:, h * 65:h * 65 + 64] = wv_g[:, h * 64:(h + 1) * 64]
            bv_aug[0, h * 65:h * 65 + 64] = bv_g[h * 64:(h + 1) * 64]
            bv_aug[0, h * 65 + 64] = 1.0

        bqk = np.stack(
            [bq_g[:128], bq_g[128:], bk_g[:128], bk_g[128:]], axis=1
        ).astype(np.float32)

        in_maps.append(
            {
                "xT": xTs[b],
                "wqk": np.ascontiguousarray(wqk_p).astype(NP_BF16),
                "wv": wv_aug.astype(NP_BF16),
                "bqk": np.ascontiguousarray(bqk),
                "bv_aug": np.broadcast_to(
                    bv_aug.astype(NP_BF16), (128, VAUG)
                ).copy(),
            }
        )
    return in_maps


def _unshard(results):
    out = np.empty((B, S, HIDDEN), dtype=np.float32)
    for c in range(N_CORES):
        b, g = divmod(c, GROUPS)
        raw = results[c]["out_raw"]  # [260, 2048]
        for h in range(HG):
            ctx = raw[h * 65:h * 65 + 64]          # [64, S]
            sums = raw[h * 65 + 64]                # [S]
            col0 = g * DG + h * HEAD
            out[b, :, col0:col0 + HEAD] = (ctx / sums).T
    return out


def kernel(**inputs):
    inputs = {k: np.asarray(v, dtype=np.float32) for k, v in inputs.items()}
    nc = _get_nc()
    in_maps = _prep_core_inputs(**inputs)
    res = run_bass_kernel_spmd(nc, in_maps, core_ids=list(range(N_CORES)))
    return _unshard(res.results)


if __name__ == "__main__":
    rng = np.random.default_rng(0)
    scale = 1.0 / np.sqrt(HIDDEN)
    ins = {
        "hidden_states": rng.standard_normal((B, S, HIDDEN), dtype=np.float32),
        "Wq": rng.standard_normal((HIDDEN, HIDDEN), dtype=np.float32) * scale,
        "bq": rng.standard_normal(HIDDEN, dtype=np.float32) * 0.01,
        "Wk": rng.standard_normal((HIDDEN, HIDDEN), dtype=np.float32) * scale,
        "bk": rng.standard_normal(HIDDEN, dtype=np.float32) * 0.01,
        "Wv": rng.standard_normal((HIDDEN, HIDDEN), dtype=np.float32) * scale,
        "bv": rng.standard_normal(HIDDEN, dtype=np.float32) * 0.01,
    }
    out = kernel(**ins)

    def ref(x, Wq, bq, Wk, bk, Wv, bv):
        q = (x @ Wq + bq).reshape(B, S, NUM_HEADS, HEAD).transpose(0, 2, 1, 3)
        k = (x @ Wk + bk).reshape(B, S, NUM_HEADS, HEAD).transpose(0, 2, 1, 3)
        v = (x @ Wv + bv).reshape(B, S, NUM_HEADS, HEAD).transpose(0, 2, 1, 3)
        s = np.einsum("bhqd,bhkd->bhqk", q, k) / np.sqrt(HEAD)
        s = s - s.max(-1, keepdims=True)
        p = np.exp(s)
        p /= p.sum(-1, keepdims=True)
        c = np.einsum("bhqk,bhkd->bhqd", p, v)
        return c.transpose(0, 2, 1, 3).reshape(B, S, HIDDEN)

    exp = ref(
        ins["hidden_states"].astype(np.float64),
        ins["Wq"].astype(np.float64), ins["bq"].astype(np.float64),
        ins["Wk"].astype(np.float64), ins["bk"].astype(np.float64),
        ins["Wv"].astype(np.float64), ins["bv"].astype(np.float64),
    )
    print("L2 rel err:", np.linalg.norm(out - exp) / np.linalg.norm(exp))
    print("max abs err:", np.abs(out - exp).max())
